# revision 38
# baseline (speedup 1.0000x reference)
"""MobileViTV2 block kernel v3 — 8 TRN2 cores, data-parallel over batch.

All-bf16 datapath (fp32 PSUM), CH=1024 t-slice chunks, patch-major token
order j = p*256 + hp*16 + wp inside each chunk. z streams through DRAM in
bf16 between sweeps.

v3 changes vs v2:
  - Depthwise conv restructured as banded block-diagonal matmuls with h in
    the partition dim: partitions=(c4,h32), 9 (dt,dw) taps x 4 c4 tile-MMs
    per 4-channel block, 3 dh taps folded into each banded stationary.
    ~3x fewer PE cycles than the 27-tap diag-matmul form. Conv output
    bounces through DRAM back to channel-major layout for pw1.
  - ln_stats: wq column merged into the ones-matmul (stationary [128,2]).
  - batch_math split into two 8-chunk groups and emitted inside the sweeps
    so the LN-stat row math for chunks 0-7 overlaps chunks 8-15 of the same
    sweep; kills the inter-sweep pipeline drains.
  - Residual adds via fused DVE scalar_tensor_tensor PSUM evacuation
    (out = psum + bias + z) instead of identity-matmuls on the PE.
  - sweep_A zsum: tensor_tensor_reduce fusion (DVE) + gpsimd for a subset;
    stat-row copies moved to ACT.
"""

import sys

sys.path.insert(0, "/opt/trn_rl_repo")
import os
import numpy as np
from contextlib import ExitStack

import concourse.bass as bass
import concourse.mybir as mybir
import concourse.tile as tile
from concourse import bacc
from concourse.bass_utils import run_bass_kernel_spmd

F32 = mybir.dt.float32
BF16 = mybir.dt.bfloat16
FP8 = mybir.dt.float8e4
DR = mybir.MatmulPerfMode.DoubleRow
AF = mybir.ActivationFunctionType
OP = mybir.AluOpType
NPBF16 = mybir.dt.np(BF16)
NPFP8 = mybir.dt.np(FP8)
USE_FP8 = bool(int(os.environ.get("KERNEL_FP8", "0")))
USE_FP8_SQ = bool(int(os.environ.get("KERNEL_FP8_SQ", "0")))
USE_FP8_V = bool(int(os.environ.get("KERNEL_FP8_V", "0")))

B, C, T, H, W = 8, 256, 16, 32, 32
D, OUTC, NBLK, FF = 384, 256, 2, 768
NTOK = T * H * W
CH = 1024
NCH = 16
EPS = 1e-5

STAGE = int(os.environ.get("KERNEL_STAGE", "3"))
SUB = int(os.environ.get("KERNEL_SUB", "9"))
SIM_SAFE = bool(int(os.environ.get("KERNEL_SIM_SAFE", "0")))

SILU = AF.Square if SIM_SAFE else AF.Silu
EXP = AF.Square if SIM_SAFE else AF.Exp

# (dt, dw) tap order: dt=1 first so the full-coverage matmul opens the
# accumulation group (sets has_written on the whole bank)
TAPS9 = [(1, 0), (1, 1), (1, 2), (0, 0), (0, 1), (0, 2), (2, 0), (2, 1), (2, 2)]


def rawap(base, dims):
    return bass.AP(tensor=base.tensor, offset=base.offset, ap=[base.ap[0]] + dims)


def bcast_row(row, n, cols):
    """DRAM row -> [n, cols] broadcast-read AP."""
    return bass.AP(tensor=row.tensor, offset=row.offset, ap=[[0, n], [1, cols]])


# patch-major <-> natural permute views (within one 1024-token t-slice)
def nat2pm_in(ps):
    return ps.rearrange("p (hp ph wp pw) -> p ph pw hp wp", hp=16, ph=2, wp=16, pw=2)


def nat2pm_out(zslice):
    return zslice.rearrange("p (ph pw hp wp) -> p ph pw hp wp", ph=2, pw=2, hp=16, wp=16)


def pm2nat_in(ps):
    return ps.rearrange("p (ph pw hp wp) -> p ph hp wp pw", ph=2, pw=2, hp=16, wp=16)


def pm2nat_out(oslice):
    return oslice.rearrange("p (hp ph wp pw) -> p ph hp wp pw", hp=16, ph=2, wp=16, pw=2)


def build():
    nc = bacc.Bacc("TRN2", target_bir_lowering=False, debug=False, num_devices=8)

    xconv = nc.dram_tensor("xconv", [64, 128, 544], BF16, kind="ExternalInput").ap()
    SwD = nc.dram_tensor("SwD", [64, 9, 128, 32], BF16, kind="ExternalInput").ap()
    dwB2 = nc.dram_tensor("dwB2", [128, 64], F32, kind="ExternalInput").ap()
    eyeD = nc.dram_tensor("eye", [128, 128], BF16, kind="ExternalInput").ap()
    sel4D = nc.dram_tensor("sel4", [64, 4], F32, kind="ExternalInput").ap()
    pw1W = nc.dram_tensor("pw1W", [C, D], BF16, kind="ExternalInput").ap()
    pw1B = nc.dram_tensor("pw1B", [128, 3], F32, kind="ExternalInput").ap()
    pw2W = nc.dram_tensor("pw2W", [D, OUTC], BF16, kind="ExternalInput").ap()
    pw2B = nc.dram_tensor("pw2B", [128, 2], F32, kind="ExternalInput").ap()
    blk = []
    for i in range(NBLK):
        blk.append(dict(
            wq=nc.dram_tensor(f"wq{i}", [D, 1], BF16, kind="ExternalInput").ap(),
            wk=nc.dram_tensor(f"wk{i}", [D, D], BF16, kind="ExternalInput").ap(),
            wv=nc.dram_tensor(f"wv{i}", [D, D], BF16, kind="ExternalInput").ap(),
            qB=nc.dram_tensor(f"qB{i}", [1, 1], F32, kind="ExternalInput").ap(),
            kB=nc.dram_tensor(f"kB{i}", [128, 3], F32, kind="ExternalInput").ap(),
            vB=nc.dram_tensor(f"vB{i}", [128, 3], F32, kind="ExternalInput").ap(),
            woW=nc.dram_tensor(f"woW{i}", [D, D], BF16, kind="ExternalInput").ap(),
            woB=nc.dram_tensor(f"woB{i}", [128, 3], F32, kind="ExternalInput").ap(),
            ff1W=nc.dram_tensor(f"ff1W{i}", [D, FF], BF16, kind="ExternalInput").ap(),
            ff1B=nc.dram_tensor(f"ff1B{i}", [128, 6], F32, kind="ExternalInput").ap(),
            ff2W=nc.dram_tensor(f"ff2W{i}", [FF, D], BF16, kind="ExternalInput").ap(),
            ff2B=nc.dram_tensor(f"ff2B{i}", [128, 3], F32, kind="ExternalInput").ap(),
            f1d=nc.dram_tensor(f"f1d{i}", [128, 2, FF], FP8, kind="ExternalInput").ap(),
            f1r=nc.dram_tensor(f"f1r{i}", [128, FF], FP8, kind="ExternalInput").ap(),
            f2d=nc.dram_tensor(f"f2d{i}", [3, 128, 2, D], FP8, kind="ExternalInput").ap(),
        ))
    qfix = nc.dram_tensor("qfix", [NBLK, 1], F32, kind="ExternalInput").ap()

    out = nc.dram_tensor("out", [OUTC, NTOK], F32, kind="ExternalOutput").ap()
    zst = [nc.dram_tensor(n, [3, 128, NTOK], BF16, kind="ExternalOutput").ap()
           for n in ("z0", "zm0", "z1", "zm1")]
    ydramT = nc.dram_tensor("ydramT", [C, NTOK], BF16).ap()  # [c, t*1024 + h*32 + w]
    VDT = FP8 if (USE_FP8 or USE_FP8_V) else BF16
    vd = [nc.dram_tensor(f"v{i}", [3, 128, NTOK], VDT).ap() for i in range(NBLK)]
    statd = [nc.dram_tensor(f"statd{e}", [3, NCH, CH], BF16).ap() for e in range(4)]
    rd = [nc.dram_tensor(f"rd{e}", [2, NCH, CH], BF16).ap() for e in range(4)]
    csd = [nc.dram_tensor(f"csd{i}", [NCH, CH], BF16).ap() for i in range(NBLK)]

    with ExitStack() as ctx:
        tc = ctx.enter_context(tile.TileContext(nc))
        wpool = ctx.enter_context(tc.tile_pool(name="w", bufs=1))
        sp = ctx.enter_context(tc.tile_pool(name="s", bufs=2))
        pp = ctx.enter_context(tc.tile_pool(name="ps", bufs=2, space="PSUM"))
        cvp = ctx.enter_context(tc.tile_pool(name="cv", bufs=1))
        p1 = ctx.enter_context(tc.tile_pool(name="p1", bufs=2))

        # ================= S0a: banded depthwise conv =================
        # program order puts these DMAs first so the PE starts ~immediately
        dwb2_t = wpool.tile([128, 64], F32, tag="dwB2")
        nc.sync.dma_start(out=dwb2_t[:], in_=dwB2)
        for cblk in range(64):
            xc = p1.tile([128, 544], BF16, tag="xc", name="xc", bufs=6)
            nc.sync.dma_start(out=xc[:], in_=xconv[cblk])
            sd = p1.tile([128, 9, 32], BF16, tag="sd", name="sd", bufs=6)
            nc.sync.dma_start(out=sd[:], in_=SwD[cblk].rearrange("n p m -> p n m"))
            psc = pp.tile([128, 512], F32, tag="psC", bufs=2)
            xcv = xc[:].rearrange("p (t w) -> p t w", t=16)
            for j, (dt, dwi) in enumerate(TAPS9):
                lo = max(0, 1 - dt)
                hi = 16 + min(0, 1 - dt)
                for c4 in range(4):
                    pa = slice(c4 * 32, c4 * 32 + 32)
                    nc.tensor.matmul(psc[pa, lo * 32 : hi * 32], sd[pa, dt * 3 + dwi, :],
                                     xcv[pa, lo + dt - 1 : hi + dt - 1, dwi : dwi + 32],
                                     start=(j == 0), stop=(j == 8),
                                     tile_position=(c4 * 32, c4 * 32))
            yc = sp.tile([128, 512], BF16, tag="yc", name="yc", bufs=3)
            nc.scalar.activation(out=yc[:], in_=psc[:], func=SILU, bias=dwb2_t[:, cblk : cblk + 1])
            # transposing store: channel-major ydramT[c, t*1024 + h*32 + w];
            # partition (c4,h) -> row h*32 offsets of channel c4*64+cblk
            ydv = ydramT.rearrange("c (t h w) -> c t h w", t=16, h=32, w=32)
            for c4 in range(4):
                src = yc[c4 * 32 : (c4 + 1) * 32, :].rearrange("p (t w) -> p t w", t=16)
                nc.sync.dma_start(out=ydv[c4 * 64 + cblk].rearrange("t h w -> h t w"), in_=src)

        # ================= weights =================
        def wt(name, dram, kdim, mdim, dt=BF16):
            tiles = []
            for ki in range((kdim + 127) // 128):
                t = wpool.tile([128, mdim], dt, tag=f"{name}{ki}")
                nc.sync.dma_start(out=t[:], in_=dram[ki * 128 : (ki + 1) * 128, :])
                tiles.append(t)
            return tiles

        def ftile(name, dram, cols):
            t = wpool.tile([128, cols], F32, tag=name)
            nc.sync.dma_start(out=t[:], in_=dram)
            return t

        def fp8t(name, dram, shape):
            if not USE_FP8:
                return None
            t = wpool.tile(shape, FP8, tag=name)
            nc.sync.dma_start(out=t[:], in_=dram)
            return t

        pw1_t = wt("pw1", pw1W, C, D)
        pw1b_t = ftile("pw1B", pw1B, 3)
        pw2_t = wt("pw2", pw2W, D, OUTC)
        pw2b_t = ftile("pw2B", pw2B, 2)
        eye_t = wpool.tile([128, 128], BF16, tag="eye")
        nc.sync.dma_start(out=eye_t[:], in_=eyeD)
        sel4_t = wpool.tile([64, 4], F32, tag="sel4")
        nc.sync.dma_start(out=sel4_t[:], in_=sel4D)
        ones_bf = wpool.tile([128, 1], BF16, tag="ones_bf")
        nc.vector.memset(ones_bf[:], 1.0)
        ones8 = wpool.tile([128, 1], FP8, tag="ones8")
        nc.vector.memset(ones8[:], 1.0)
        ones8d = wpool.tile([128, 2, 1], FP8, tag="ones8d")
        nc.vector.memset(ones8d[:], 1.0)
        bw = []
        for i in range(NBLK):
            owq = []
            for kt in range(3):
                t = wpool.tile([128, 2], BF16, tag=f"owq{i}_{kt}", name=f"owq{i}_{kt}")
                nc.vector.memset(t[:, 0:1], 1.0)
                nc.sync.dma_start(out=t[:, 1:2], in_=blk[i]["wq"][kt * 128 : (kt + 1) * 128, :])
                owq.append(t)
            bw.append(dict(
                owq=owq,
                wk=wt(f"wk{i}_", blk[i]["wk"], D, D),
                wv=wt(f"wv{i}_", blk[i]["wv"], D, D),
                kB=ftile(f"kB{i}", blk[i]["kB"], 3),
                vB=ftile(f"vB{i}", blk[i]["vB"], 3),
                wo=wt(f"wo{i}_", blk[i]["woW"], D, D),
                woB=ftile(f"woB{i}", blk[i]["woB"], 3),
                ff1=(wt(f"ff1{i}_", blk[i]["ff1W"], D, FF) if not USE_FP8 else None),
                ff1B=ftile(f"ff1B{i}", blk[i]["ff1B"], 6),
                ff2=(wt(f"ff2{i}_", blk[i]["ff2W"], FF, D) if not USE_FP8 else None),
                ff2B=ftile(f"ff2B{i}", blk[i]["ff2B"], 3),
                f1d=fp8t(f"f1d{i}", blk[i]["f1d"], [128, 2, FF]),
                f1r=fp8t(f"f1r{i}", blk[i]["f1r"], [128, FF]),
                f2d=[fp8t(f"f2d{i}_{j}", blk[i]["f2d"][j], [128, 2, D]) for j in range(3)],
            ))
        ones64f = wpool.tile([64, 1], F32, tag="ones64f")
        nc.vector.memset(ones64f[:], 1.0)
        ones1r = wpool.tile([1, 128], F32, tag="ones1r")
        nc.vector.memset(ones1r[:], 1.0)
        eps64 = wpool.tile([64, 1], F32, tag="eps64")
        nc.vector.memset(eps64[:], EPS)
        qb64 = [wpool.tile([64, 1], F32, tag=f"qb64_{i}", name=f"qb64_{i}") for i in range(NBLK)]
        sq64 = [wpool.tile([64, 1], F32, tag=f"sq64_{i}", name=f"sq64_{i}") for i in range(NBLK)]
        for i in range(NBLK):
            nc.sync.dma_start(out=qb64[i][:], in_=bcast_row(blk[i]["qB"][0, :], 64, 1))
            nc.sync.dma_start(out=sq64[i][:], in_=bcast_row(qfix[i, :], 64, 1))

        wop = [cvp.tile([128, 4, 384], BF16, tag=f"wop{k}", name=f"wop{k}") for k in range(3)]
        att = []
        for i in range(NBLK):
            a = dict(
                cvacc=cvp.tile([128, 3, 4], F32, tag=f"cvacc{i}", name=f"cvacc{i}"),
                zinvb=cvp.tile([128, 4], F32, tag=f"zinvb{i}", name=f"zinvb{i}"),
                zp=[cvp.tile([64, 1], F32, tag=f"zp{i}_{g}", name=f"zp{i}_{g}") for g in range(2)],
                wop=wop,
            )
            nc.vector.memset(a["cvacc"][:], 0.0)
            att.append(a)

        # ---------- helpers ----------
        def ln_stats(zt, owq_tiles, event, chunk, sq_act, sb_act=True):
            zsq = sp.tile([128, 3, CH], FP8 if USE_FP8_SQ else BF16, tag="zsq", name="zsq", bufs=1)
            if sq_act:
                nc.scalar.activation(out=zsq[:], in_=zt[:], func=AF.Square)
            else:
                nc.vector.tensor_mul(zsq[:], zt[:], zt[:])
            ps = pp.tile([128, CH], F32, tag="pstat", bufs=1)
            for half in range(2):
                hsl = slice(half * 512, (half + 1) * 512)
                for kt in range(3):
                    if owq_tiles is not None:
                        nc.tensor.matmul(ps[0:2, hsl], owq_tiles[kt][:], zt[:, kt, hsl], start=(kt == 0), stop=(kt == 2))
                    else:
                        nc.tensor.matmul(ps[0:1, hsl], ones_bf[:], zt[:, kt, hsl], start=(kt == 0), stop=(kt == 2))
                if USE_FP8_SQ:
                    nc.tensor.matmul(ps[32:33, hsl], ones8d[:], zsq[:, 0:2, hsl], start=True, stop=False, perf_mode=DR)
                    nc.tensor.matmul(ps[32:33, hsl], ones8[:], zsq[:, 2, hsl], start=False, stop=True)
                else:
                    for kt in range(3):
                        nc.tensor.matmul(ps[32:33, hsl], ones_bf[:], zsq[:, kt, hsl], start=(kt == 0), stop=(kt == 2))
            sb = sp.tile([33, CH], BF16, tag="sbstat", name="sbstat", bufs=2)
            if sb_act:
                nc.scalar.activation(out=sb[:], in_=ps[0:33, :], func=AF.Copy)
            else:
                nc.vector.tensor_copy(sb[:], ps[0:33, :])
            nc.sync.dma_start(out=statd[event][0, chunk, :], in_=sb[0:1, :])
            nc.sync.dma_start(out=statd[event][1, chunk, :], in_=sb[32:33, :])
            if owq_tiles is not None:
                nc.sync.dma_start(out=statd[event][2, chunk, :], in_=sb[1:2, :])

        def batch_math(event, bi, g):
            with nc.allow_low_precision(reason="LN stat math in bf16 is within error budget"):
                return _batch_math(event, bi, g)

        def _batch_math(event, bi, g):
            # stat rows repacked [64 partitions, 128]: partition = (chunk, b),
            # token inside chunk = b*128 + t  ->  8x faster DVE row math
            gs = slice(g * 64, (g + 1) * 64)
            sdv = statd[event].rearrange("s n (b t) -> s (n b) t", b=8)
            bs = sp.tile([64, 3, 128], BF16, tag="bs", name="bs", bufs=1)
            for s in range(3 if bi is not None else 2):
                nc.sync.dma_start(out=bs[:, s, :], in_=sdv[s, gs, :])
            tmp = sp.tile([64, 5, 128], BF16, tag="bstmp", name="bstmp", bufs=1)
            M, t2, R, MR, q1 = (tmp[:, j, :] for j in range(5))
            nc.vector.tensor_scalar_mul(out=M, in0=bs[:, 0, :], scalar1=1.0 / D)
            nc.vector.tensor_mul(t2, M, M)
            nc.vector.scalar_tensor_tensor(out=t2, in0=bs[:, 1, :], scalar=1.0 / D, in1=t2, op0=OP.mult, op1=OP.subtract)
            nc.scalar.activation(out=t2, in_=t2, func=AF.Sqrt, bias=eps64[:])
            nc.vector.reciprocal(R, t2)
            nc.vector.tensor_mul(MR, M, R)
            rb = sp.tile([64, 2, 128], BF16, tag="rbf", name="rbf", bufs=1)
            nc.vector.tensor_copy(rb[:, 0, :], R)
            nc.vector.tensor_copy(rb[:, 1, :], MR)
            rdv = rd[event].rearrange("r n (b t) -> r (n b) t", b=8)
            nc.sync.dma_start(out=rdv[0, gs, :], in_=rb[:, 0, :])
            nc.sync.dma_start(out=rdv[1, gs, :], in_=rb[:, 1, :])
            if bi is None:
                return
            nc.vector.scalar_tensor_tensor(out=q1, in0=M, scalar=sq64[bi][:], in1=bs[:, 2, :], op0=OP.mult, op1=OP.add)
            nc.vector.tensor_mul(q1, q1, R)
            cs = sp.tile([64, 128], BF16, tag="bscs", name="bscs", bufs=1)
            zp = att[bi]["zp"][g]
            nc.scalar.activation(out=cs[:], in_=q1[:], func=EXP, bias=qb64[bi][:], accum_out=zp[:, 0:1])
            csdv = csd[bi].rearrange("n (b t) -> (n b) t", b=8)
            nc.sync.dma_start(out=csdv[gs, :], in_=cs[:])
            if g == 1:
                rhs = sp.tile([64, 2, 4], F32, tag="zrhs", name="zrhs", bufs=1)
                nc.vector.tensor_scalar_mul(out=rhs[:, 0, :], in0=sel4_t[:], scalar1=att[bi]["zp"][0][:, 0:1])
                nc.vector.tensor_scalar_mul(out=rhs[:, 1, :], in0=sel4_t[:], scalar1=att[bi]["zp"][1][:, 0:1])
                psz = pp.tile([128, CH], F32, tag="pstat", bufs=1)
                nc.tensor.matmul(psz[0:1, 0:4], ones64f[:], rhs[:, 0, :], start=True, stop=False)
                nc.tensor.matmul(psz[0:1, 0:4], ones64f[:], rhs[:, 1, :], start=False, stop=True)
                zi = sp.tile([1, 4], F32, tag="zi", name="zi", bufs=1)
                nc.vector.reciprocal(zi[:], psz[0:1, 0:4])
                psb = pp.tile([128, CH], F32, tag="pstat", bufs=1)
                nc.tensor.matmul(psb[:, 0:4], ones1r[:], zi[:], start=True, stop=True)
                nc.vector.tensor_copy(att[bi]["zinvb"][:], psb[:, 0:4])

        def load_bcast(dram_row):
            t = sp.tile([128, CH], BF16, tag="bcast", name="bcast", bufs=3)
            nc.sync.dma_start(out=t[:], in_=bcast_row(dram_row, 128, CH))
            return t

        def load_z(dram, chunk, tag="zch", dt=BF16):
            zt = sp.tile([128, 3, CH], dt, tag=tag, name=tag, bufs=2)
            for m in range(3):
                nc.sync.dma_start(out=zt[:, m, :], in_=dram[m, :, chunk * CH : (chunk + 1) * CH])
            return zt

        def normalize(zt, event, chunk, dt=BF16):
            row01 = rd[event][:, chunk, :]
            rmr = sp.tile([128, 2, CH], BF16, tag="rmr", name="rmr", bufs=3)
            nc.sync.dma_start(out=rmr[:], in_=bass.AP(tensor=row01.tensor, offset=row01.offset,
                                                      ap=[[0, 128]] + row01.ap))
            zn = sp.tile([128, 3, CH], dt, tag="zn" + ("8" if dt is FP8 else ""), name="zn", bufs=2)
            nc.vector.tensor_mul(zn[:], zt[:], rawap(rmr[:, 0, :], [[0, 3], [1, CH]]))
            nc.vector.tensor_sub(zn[:], zn[:], rawap(rmr[:, 1, :], [[0, 3], [1, CH]]))
            return zn

        # ================= S0b: reload conv output, pw1, LN1_0 stats =================
        for t in range(T):
            yact = p1.tile([128, 2, CH], BF16, tag="yact", name="yact", bufs=4)
            for cti in range(2):
                nc.sync.dma_start(out=yact[:, cti, :],
                                  in_=ydramT[cti * 128 : (cti + 1) * 128, t * CH : (t + 1) * CH])
            zt = sp.tile([128, 3, CH], BF16, tag="zch", name="zch", bufs=2)
            for m in range(3):
                ps1 = pp.tile([128, CH], F32, tag="psA", bufs=2)
                for half in range(2):
                    hsl = slice(half * 512, (half + 1) * 512)
                    for kt in range(2):
                        nc.tensor.matmul(ps1[:, hsl], pw1_t[kt][:, m * 128 : (m + 1) * 128], yact[:, kt, hsl], start=(kt == 0), stop=(kt == 1))
                for ph_ in range(2):
                    nc.scalar.activation(out=nat2pm_out(zt[:, m, :])[:, ph_], in_=nat2pm_in(ps1[:])[:, ph_], func=AF.Identity, bias=pw1b_t[:, m : m + 1])
                nc.sync.dma_start(out=zst[0][m, :, t * CH : (t + 1) * CH], in_=zt[:, m, :])
            ln_stats(zt, bw[0]["owq"], 0, t, sq_act=False, sb_act=False)
            if t == 7 and STAGE >= 2:
                batch_math(0, 0, 0)
        if STAGE >= 2:
            batch_math(0, 0, 1)

        # ================= per-block sweeps =================
        def sweep_A(bi, zsrc, event):
            a = att[bi]
            for chunk in range(NCH):
                zt = load_z(zsrc, chunk)
                zn = normalize(zt, event, chunk)
                csb = load_bcast(csd[bi][chunk, :])
                vt = sp.tile([128, 3, CH], VDT, tag="vch", name="vch", bufs=2)
                for m in range(3):
                    psv = pp.tile([128, CH], F32, tag="psA", bufs=2)
                    for half in range(2):
                        hsl = slice(half * 512, (half + 1) * 512)
                        for kt in range(3):
                            nc.tensor.matmul(psv[:, hsl], bw[bi]["wv"][kt][:, m * 128 : (m + 1) * 128], zn[:, kt, hsl], start=(kt == 0), stop=(kt == 2))
                    nc.scalar.activation(out=vt[:, m, :], in_=psv[:], func=AF.Relu, bias=bw[bi]["vB"][:, m : m + 1])
                    nc.sync.dma_start(out=vd[bi][m, :, chunk * CH : (chunk + 1) * CH], in_=vt[:, m, :])
                junk = sp.tile([128, 3, CH], BF16, tag="junk", name="junk", bufs=2)
                csb3 = rawap(csb[:], [[0, 3], [1, CH]])
                nc.gpsimd.tensor_mul(junk[:], zn[:], csb3)
                cvch = sp.tile([128, 3, 4], F32, tag="cvch", name="cvch", bufs=2)
                for m in range(3):
                    for p in range(4):
                        if (m + p) % 2 == 0:
                            nc.vector.tensor_reduce(cvch[:, m, p : p + 1], junk[:, m, p * 256 : (p + 1) * 256],
                                                    axis=mybir.AxisListType.X, op=OP.add)
                        else:
                            nc.scalar.activation(out=junk[:, m, p * 256 : (p + 1) * 256], in_=junk[:, m, p * 256 : (p + 1) * 256],
                                                 func=AF.Copy, accum_out=cvch[:, m, p : p + 1])
                nc.vector.tensor_add(a["cvacc"][:], a["cvacc"][:], cvch[:])

        def finalize_cv(bi):
            a = att[bi]
            cvb = sp.tile([128, 3, 4], BF16, tag="cvb", name="cvb", bufs=1)
            nc.vector.tensor_copy(cvb[:], a["cvacc"][:])
            cvf = sp.tile([128, 3, 4], F32, tag="cvf", name="cvf", bufs=1)
            for m in range(3):
                psc2 = pp.tile([128, CH], F32, tag="pstat", bufs=1)
                for kt in range(3):
                    nc.tensor.matmul(psc2[:, 0:4], bw[bi]["wk"][kt][:, m * 128 : (m + 1) * 128], cvb[:, kt, :], start=(kt == 0), stop=(kt == 2))
                nc.vector.scalar_tensor_tensor(out=cvf[:, m, :], in0=psc2[:, 0:4], scalar=bw[bi]["kB"][:, m : m + 1], in1=a["zinvb"][:], op0=OP.add, op1=OP.mult)
            for kt in range(3):
                for p in range(4):
                    nc.vector.tensor_scalar_mul(out=a["wop"][kt][:, p, :], in0=bw[bi]["wo"][kt][:], scalar1=cvf[:, kt, p : p + 1])

        def sweep_B(bi, zsrc, zdst, event):
            a = att[bi]
            for chunk in range(NCH):
                zt = load_z(zsrc, chunk)
                vt = sp.tile([128, 3, CH], VDT, tag="vch", name="vch", bufs=2)
                for m in range(3):
                    nc.sync.dma_start(out=vt[:, m, :], in_=vd[bi][m, :, chunk * CH : (chunk + 1) * CH])
                zm = sp.tile([128, 3, CH], BF16, tag="zm", name="zm", bufs=2)
                for m in range(3):
                    pso = pp.tile([128, CH], F32, tag="psA", bufs=2)
                    # residual preloaded into PSUM by ACT; matmuls accumulate on top
                    # (has_written bits are set from the bank's previous full-width group)
                    nc.scalar.activation(out=pso[:], in_=zt[:, m, :], func=AF.Copy)
                    for p in range(4):
                        sl = slice(p * 256, (p + 1) * 256)
                        for kt in range(3):
                            nc.tensor.matmul(pso[:, sl], a["wop"][kt][:, p, m * 128 : (m + 1) * 128], vt[:, kt, sl], start=False, stop=(kt == 2), skip_group_check=True)
                    nc.scalar.activation(out=zm[:, m, :], in_=pso[:], func=AF.Identity, bias=bw[bi]["woB"][:, m : m + 1])
                    nc.sync.dma_start(out=zdst[m, :, chunk * CH : (chunk + 1) * CH], in_=zm[:, m, :])
                ln_stats(zm, None, event, chunk, sq_act=False, sb_act=True)
                if chunk == 7:
                    batch_math(event, None, 0)
            batch_math(event, None, 1)

        def sweep_F(bi, zsrc, zdst, event_in, event_out, owq_next, bi_next, last):
            for chunk in range(NCH):
                zt = load_z(zsrc, chunk)
                zn = normalize(zt, event_in, chunk, dt=(FP8 if USE_FP8 else BF16))
                z2 = sp.tile([128, 3, CH], BF16, tag="z2", name="z2", bufs=2)
                ht = sp.tile([128, 6, CH], FP8 if USE_FP8 else BF16, tag="ht", name="ht", bufs=1)
                for m6 in range(6):
                    ps1 = pp.tile([128, CH], F32, tag="psA", bufs=2)
                    for half in range(2):
                        hsl = slice(half * 512, (half + 1) * 512)
                        if USE_FP8:
                            nc.tensor.matmul(ps1[:, hsl], bw[bi]["f1d"][:, :, m6 * 128 : (m6 + 1) * 128], zn[:, 0:2, hsl], start=True, stop=False, perf_mode=DR)
                            nc.tensor.matmul(ps1[:, hsl], bw[bi]["f1r"][:, m6 * 128 : (m6 + 1) * 128], zn[:, 2, hsl], start=False, stop=True)
                        else:
                            for kt in range(3):
                                nc.tensor.matmul(ps1[:, hsl], bw[bi]["ff1"][kt][:, m6 * 128 : (m6 + 1) * 128], zn[:, kt, hsl], start=(kt == 0), stop=(kt == 2))
                    nc.scalar.activation(out=ht[:, m6, :], in_=ps1[:], func=SILU, bias=bw[bi]["ff1B"][:, m6 : m6 + 1])
                for m in range(3):
                    psf = pp.tile([128, CH], F32, tag="psA", bufs=2)
                    nc.scalar.activation(out=psf[:], in_=zt[:, m, :], func=AF.Copy)
                    for half in range(2):
                        hsl = slice(half * 512, (half + 1) * 512)
                        if USE_FP8:
                            for j in range(3):
                                nc.tensor.matmul(psf[:, hsl], bw[bi]["f2d"][j][:, :, m * 128 : (m + 1) * 128], ht[:, 2 * j : 2 * j + 2, hsl], start=False, stop=(j == 2), perf_mode=DR, skip_group_check=True)
                        else:
                            for m6 in range(6):
                                nc.tensor.matmul(psf[:, hsl], bw[bi]["ff2"][m6][:, m * 128 : (m + 1) * 128], ht[:, m6, hsl], start=False, stop=(m6 == 5), skip_group_check=True)
                    nc.scalar.activation(out=z2[:, m, :], in_=psf[:], func=AF.Identity, bias=bw[bi]["ff2B"][:, m : m + 1])
                    if zdst is not None:
                        nc.sync.dma_start(out=zdst[m, :, chunk * CH : (chunk + 1) * CH], in_=z2[:, m, :])
                if not last:
                    ln_stats(z2, owq_next, event_out, chunk, sq_act=False, sb_act=True)
                    if chunk == 7:
                        batch_math(event_out, bi_next, 0)
                else:
                    for m in range(2):
                        ot = sp.tile([128, CH], F32, tag="ot", name="ot", bufs=1)
                        ps2 = pp.tile([128, CH], F32, tag="pstat", bufs=1)
                        for half in range(2):
                            hsl = slice(half * 512, (half + 1) * 512)
                            for kt in range(3):
                                nc.tensor.matmul(ps2[:, hsl], pw2_t[kt][:, m * 128 : (m + 1) * 128], z2[:, kt, hsl], start=(kt == 0), stop=(kt == 2))
                        for ph_ in range(2):
                            nc.scalar.activation(out=pm2nat_out(ot[:])[:, ph_], in_=pm2nat_in(ps2[:])[:, ph_], func=AF.Identity, bias=pw2b_t[:, m : m + 1])
                        nc.sync.dma_start(out=out[m * 128 : (m + 1) * 128, chunk * CH : (chunk + 1) * CH], in_=ot[:])
            if not last:
                batch_math(event_out, bi_next, 1)

        if STAGE >= 2:
            if SUB >= 2:
                sweep_A(0, zst[0], 0)
            if SUB >= 3:
                finalize_cv(0)
            if SUB >= 4:
                sweep_B(0, zst[0], zst[1], 1)
            if SUB >= 5:
                sweep_F(0, zst[1], zst[2], 1, 2, bw[1]["owq"], 1, last=False)
        if STAGE >= 3:
            sweep_A(1, zst[2], 2)
            finalize_cv(1)
            sweep_B(1, zst[2], zst[3], 3)
            sweep_F(1, zst[3], None, 3, None, None, None, last=True)

    nc.compile()
    return nc


_NC = None


def _get_nc():
    global _NC
    if _NC is None:
        _NC = build()
    return _NC


def _prep(inputs):
    f32 = lambda a: np.ascontiguousarray(np.asarray(a, np.float32))
    bf = lambda a: np.ascontiguousarray(np.asarray(a, np.float32)).astype(NPBF16)
    dw27 = f32(inputs["dw_w"]).reshape(C, 27)
    dwf = dw27.reshape(4, 64, 3, 3, 3)  # [c4, cblk, dt, dh, dw], c = c4*64+cblk
    Sd = np.zeros((64, 9, 128, 32), np.float32)
    for c4 in range(4):
        for dh in range(3):
            off = dh - 1
            hout = np.arange(max(0, -off), 32 - max(0, off))
            hin = hout + off
            val = dwf[c4, :, :, dh, :].reshape(64, 9)
            Sd[:, :, c4 * 32 + hin, hout] = val[:, :, None]
    dwb2 = np.broadcast_to(f32(inputs["dw_b"]).reshape(4, 1, 64), (4, 32, 64)).reshape(128, 64)
    base = {
        "SwD": Sd.astype(NPBF16),
        "dwB2": np.ascontiguousarray(dwb2),
        "eye": np.eye(128, dtype=np.float32).astype(NPBF16),
        "sel4": np.ascontiguousarray((((np.arange(64) % 8)[:, None] // 2) == np.arange(4)[None, :]).astype(np.float32)),
        "pw1W": bf(inputs["pw1_w"]),
        "pw1B": np.ascontiguousarray(f32(inputs["pw1_b"]).reshape(3, 128).T),
        "pw2W": bf(inputs["pw2_w"]),
        "pw2B": np.ascontiguousarray(f32(inputs["pw2_b"]).reshape(2, 128).T),
    }
    qf = np.zeros((NBLK, 1), np.float32)
    for i in range(NBLK):
        qkvW = f32(inputs["ln1_g"][i])[:, None] * f32(inputs["qkv_w"][i])
        qkvB = f32(inputs["ln1_b"][i]) @ f32(inputs["qkv_w"][i]) + f32(inputs["qkv_b"][i])
        ff1W = f32(inputs["ln2_g"][i])[:, None] * f32(inputs["ff1_w"][i])
        ff1B = f32(inputs["ln2_b"][i]) @ f32(inputs["ff1_w"][i]) + f32(inputs["ff1_b"][i])
        wqb = np.ascontiguousarray(qkvW[:, 0:1]).astype(NPBF16)
        qf[i, 0] = -float(np.asarray(wqb, np.float32).sum())
        base.update({
            f"wq{i}": wqb,
            f"wk{i}": np.ascontiguousarray(qkvW[:, 1 : 1 + D]).astype(NPBF16),
            f"wv{i}": np.ascontiguousarray(qkvW[:, 1 + D :]).astype(NPBF16),
            f"qB{i}": np.ascontiguousarray(qkvB[0:1].reshape(1, 1)),
            f"kB{i}": np.ascontiguousarray(qkvB[1 : 1 + D].reshape(3, 128).T),
            f"vB{i}": np.ascontiguousarray(qkvB[1 + D :].reshape(3, 128).T),
            f"woW{i}": bf(inputs["wo_w"][i]),
            f"woB{i}": np.ascontiguousarray(f32(inputs["wo_b"][i]).reshape(3, 128).T),
            f"ff1W{i}": ff1W.astype(NPBF16),
            f"ff1B{i}": np.ascontiguousarray(ff1B.reshape(6, 128).T),
            f"ff2W{i}": bf(inputs["ff2_w"][i]),
            f"ff2B{i}": np.ascontiguousarray(f32(inputs["ff2_b"][i]).reshape(3, 128).T),
            # fp8 DoubleRow layouts: plane ko holds k-tile rows [ko*128, (ko+1)*128)
            f"f1d{i}": np.ascontiguousarray(ff1W[:256].reshape(2, 128, FF).transpose(1, 0, 2)).astype(NPFP8),
            f"f1r{i}": np.ascontiguousarray(ff1W[256:]).astype(NPFP8),
            f"f2d{i}": np.ascontiguousarray(f32(inputs["ff2_w"][i]).reshape(3, 2, 128, D).transpose(0, 2, 1, 3)).astype(NPFP8),
        })
    base["qfix"] = qf
    return base


def kernel(**inputs):
    base = _prep(inputs)
    x = np.asarray(inputs["x"], np.float32)
    in_maps = []
    for b in range(B):
        xv = x[b].reshape(4, 64, T, H, W)
        xp = np.zeros((64, 4, 32, 16, 34), np.float32)
        xp[:, :, :, :, 1:33] = xv.transpose(1, 0, 3, 2, 4)
        in_maps.append(dict(base, xconv=xp.reshape(64, 128, 544).astype(NPBF16)))
    nc = _get_nc()
    trace = bool(int(os.environ.get("KERNEL_TRACE", "0")))
    res = run_bass_kernel_spmd(nc, in_maps, list(range(B)), trace=trace)
    kernel.last_exec_ns = res.exec_time_ns
    kernel.last_profile = res.profile_json
    kernel.last_results = res.results
    outs = [res.results[b]["out"].reshape(OUTC, T, H, W) for b in range(B)]
    return np.stack(outs).astype(np.float32)
